# revision 1
# baseline (speedup 1.0000x reference)
"""Trainium2 Bass kernel for nn_Discriminator_lstm (B=4096, T=32, E=H=300, VOCAB=10000).

Strategy (data-parallel over batch, 8 cores x 512 rows):
  Phase 1 (per core, replicated): G = embed_w @ W_ih^T  -> DRAM scratch [10000, 1280]
          (input projection folded into an embedding-table transform; the
           per-token input projection then becomes a row *gather* of G).
  Phase 2: 32-step LSTM recurrence.  Per step:
          - indirect-DMA gather xg = G[cap[:, t]] into SBUF
          - PE: inject xg into PSUM (identity matmul), accumulate
            gates += [h | 1] @ [W_hh^T ; b]  (bias rides an ones-row)
          - ACT: sigmoid on [i f o] (contiguous after gate reorder), tanh on g
          - DVE: c = f*c + i*g ; h = o*tanh(c); masked capture of h_last
          - PE transpose h -> hT for the next step's stationary operand
  Phase 3: logits = h_last @ Wc_norm^T + b  (weight-normed classifier, bias
          rides the ones-row again), DMA out.

Matmul inputs run as float32r (full PE rate at N>=256) or bf16 (MM_DT below).
"""

import os
import sys

import numpy as np

for _p in ("/opt/trn_rl_repo", "/root/.axon_site/_ro/trn_rl_repo"):
    if os.path.isdir(_p) and _p not in sys.path:
        sys.path.insert(0, _p)

import concourse.bass as bass
import concourse.bacc as bacc
import concourse.mybir as mybir
import concourse.tile as tile
from concourse.bass_utils import run_bass_kernel_spmd
from concourse.masks import make_identity

f32 = mybir.dt.float32
f32r = mybir.dt.float32r
bf16 = mybir.dt.bfloat16
i32 = mybir.dt.int32
u8 = mybir.dt.uint8

B, T, V, E, H = 4096, 32, 10000, 300, 300
NCORES = 8
BC = B // NCORES          # 512 batch rows per core
M = BC // 128             # 4 m-tiles
GATE_COLS = 1280          # 4*300 gates padded to 1280 (psum chunks all >= 256)
CHUNKS = [(0, 512), (512, 1024), (1024, 1280)]
E_SPLITS = [(0, 128), (128, 256), (256, 300)]       # k-tiles of the E contraction
K_SPLITS = [(0, 128), (128, 256), (256, 301)]       # k-tiles of the [h|1] contraction
H_SPLITS = [(0, 128), (128, 256), (256, 300)]       # h-dim splits for transposes
VTILES = (V + 127) // 128                            # 79

MM_DT = os.environ.get("KERNEL_MM_DT", "f32r")       # "f32r" | "bf16"
G_DT = os.environ.get("KERNEL_G_DT", MM_DT)          # "f32r" | "bf16"


def _raw(inst):
    return getattr(inst, "ins", inst)


def _set_row(nc, ap, row, value):
    """Set one partition row of an already-zeroed tile to `value`.

    Engine APs cannot start at arbitrary partitions, so write via
    affine_select over the whole tile: p == row ? value : in_.
    """
    nc.gpsimd.affine_select(
        out=ap,
        in_=ap,
        compare_op=mybir.AluOpType.not_equal,
        fill=value,
        base=-row,
        pattern=[[0, ap.shape[-1]]],
        channel_multiplier=1,
    )


def _mm(ap, mode):
    """Matmul operands are already declared in the matmul dtype."""
    return ap


def build_bass():
    mm_np_dt = f32r if MM_DT == "f32r" else bf16
    g_np_dt = f32r if G_DT == "f32r" else bf16

    nc = bacc.Bacc("TRN2", target_bir_lowering=False, debug=False, num_devices=NCORES)

    embT_d = nc.dram_tensor("embT", [E, V], mm_np_dt, kind="ExternalInput")
    wih_d = nc.dram_tensor("wih", [E, GATE_COLS], mm_np_dt, kind="ExternalInput")
    waug_d = nc.dram_tensor("waug", [H + 1, GATE_COLS], mm_np_dt, kind="ExternalInput")
    wc_d = nc.dram_tensor("wc", [H + 1, 2], mm_np_dt, kind="ExternalInput")
    ident_d = nc.dram_tensor("ident", [128, 128], mm_np_dt, kind="ExternalInput")
    identg_d = nc.dram_tensor("identg", [128, 128], g_np_dt, kind="ExternalInput")
    hT_init_d = nc.dram_tensor("hT_init", [H + 1, BC], mm_np_dt, kind="ExternalInput")
    idx_d = nc.dram_tensor("idx", [128, T, M], i32, kind="ExternalInput")
    mask_d = nc.dram_tensor("mask", [128, T, M], u8, kind="ExternalInput")
    G_d = nc.dram_tensor("G", [V, GATE_COLS], g_np_dt, kind="Internal")
    out_d = nc.dram_tensor("out", [BC, 2], f32, kind="ExternalOutput")

    with tile.TileContext(nc, num_cores=NCORES) as tc:
        with (
            tc.tile_pool(name="const", bufs=1) as cpool,
            tc.tile_pool(name="state", bufs=1) as spool,
            tc.tile_pool(name="psum", bufs=2, space="PSUM") as ppool,
        ):
            # ---------- constants ----------
            ident = cpool.tile([128, 128], mm_np_dt, tag="ident")
            nc.sync.dma_start(out=ident[:, :], in_=ident_d[:, :])
            identg = ident
            if g_np_dt != mm_np_dt:
                identg = cpool.tile([128, 128], g_np_dt, tag="identg")
                nc.sync.dma_start(out=identg[:, :], in_=identg_d[:, :])

            waug_sb = []
            for k, (d0, d1) in enumerate(K_SPLITS):
                t_ = cpool.tile([d1 - d0, GATE_COLS], mm_np_dt, tag=f"waug{k}")
                nc.sync.dma_start(out=t_[:, :], in_=waug_d[d0:d1, :])
                waug_sb.append(t_)
            wc_sb = []
            for k, (d0, d1) in enumerate(K_SPLITS):
                t_ = cpool.tile([d1 - d0, 2], mm_np_dt, tag=f"wc{k}")
                nc.sync.dma_start(out=t_[:, :], in_=wc_d[d0:d1, :])
                wc_sb.append(t_)
            idx_sb = cpool.tile([128, T, M], i32, tag="idx")
            nc.sync.dma_start(out=idx_sb[:, :, :], in_=idx_d[:, :, :])
            mask_sb = cpool.tile([128, T, M], u8, tag="mask")
            nc.sync.dma_start(out=mask_sb[:, :, :], in_=mask_d[:, :, :])

            # ---------- state ----------
            hT = []
            for k, (d0, d1) in enumerate(K_SPLITS):
                t_ = spool.tile([d1 - d0, BC], mm_np_dt, tag=f"hT{k}")
                nc.sync.dma_start(out=t_[:, :], in_=hT_init_d[d0:d1, :])
                hT.append(t_)
            c_sb = spool.tile([128, M, H], f32, tag="c")
            nc.gpsimd.memset(c_sb[:, :, :], 0.0)
            # h_last needs no init: every row is written at exactly one step
            h_last = spool.tile([128, M, H], f32 if MM_DT == "f32r" else mm_np_dt, tag="hlast")
            hlT = []
            for k, (d0, d1) in enumerate(K_SPLITS):
                t_ = spool.tile([d1 - d0, BC], mm_np_dt, tag=f"hlT{k}")
                nc.sync.dma_start(out=t_[:, :], in_=hT_init_d[d0:d1, :])
                hlT.append(t_)

            # ---------- phase 1: G = embT^T @ wih ----------
            g_stores = []
            with tc.tile_pool(name="gphase", bufs=1) as gpool, \
                 tc.tile_pool(name="gsbp", bufs=3) as gsbp:
                wih_sb = []
                for k, (d0, d1) in enumerate(E_SPLITS):
                    t_ = gpool.tile([d1 - d0, GATE_COLS], mm_np_dt, tag=f"wih{k}")
                    nc.sync.dma_start(out=t_[:, :], in_=wih_d[d0:d1, :])
                    wih_sb.append(t_)
                embT_sb = []
                for k, (d0, d1) in enumerate(E_SPLITS):
                    t_ = gpool.tile([d1 - d0, V], mm_np_dt, tag=f"emb{k}")
                    for q in range(4):
                        eng = nc.sync if q % 2 == 0 else nc.scalar
                        eng.dma_start(out=t_[:, q * (V // 4):(q + 1) * (V // 4)],
                                      in_=embT_d[d0:d1, q * (V // 4):(q + 1) * (V // 4)])
                    embT_sb.append(t_)

                for v in range(VTILES):
                    rows = min(128, V - v * 128)
                    gps = ppool.tile([128, GATE_COLS], f32, tag="gates")
                    for k, (d0, d1) in enumerate(E_SPLITS):
                        for (c0, c1) in CHUNKS:
                            nc.tensor.matmul(
                                gps[0:rows, c0:c1],
                                lhsT=_mm(embT_sb[k][:, v * 128:v * 128 + rows], MM_DT),
                                rhs=_mm(wih_sb[k][:, c0:c1], MM_DT),
                                start=(k == 0),
                                stop=(k == 2),
                            )
                    gsb = gsbp.tile([128, GATE_COLS], g_np_dt, tag="gsb")
                    if v % 2 == 0:
                        nc.vector.tensor_copy(gsb[0:rows, :], gps[0:rows, :])
                    else:
                        nc.scalar.copy(gsb[0:rows, :], gps[0:rows, :])
                    eng = nc.sync if v % 2 == 0 else nc.scalar
                    st = eng.dma_start(
                        out=G_d[v * 128:v * 128 + rows, :], in_=gsb[0:rows, :]
                    )
                    g_stores.append(st)

            g_done = nc.gpsimd.nop()
            for st in g_stores:
                tile.add_dep_helper(_raw(g_done), _raw(st), reason="G stored")

            # ---------- phase 2: recurrence ----------
            with tc.tile_pool(name="work", bufs=2) as wpool:
                for t in range(T):
                    xg = wpool.tile([128, M, GATE_COLS], g_np_dt, tag="xg")
                    for m in range(M):
                        gather = nc.gpsimd.indirect_dma_start(
                            out=xg[:, m, :],
                            out_offset=None,
                            in_=G_d[:, :],
                            in_offset=bass.IndirectOffsetOnAxis(
                                ap=idx_sb[:, t, m:m + 1], axis=0),
                        )
                        tile.add_dep_helper(_raw(gather), _raw(g_done),
                                            reason="gather after G")

                    ifo = wpool.tile([128, M, 900], f32, tag="ifo")
                    gt = wpool.tile([128, M, 300], f32, tag="gt")
                    tch = wpool.tile([128, M, H], f32, tag="tch")
                    h_sb = wpool.tile([128, M, H],
                                      f32 if MM_DT == "f32r" else mm_np_dt, tag="h")
                    t1 = wpool.tile([128, M, H], f32, tag="t1")
                    ident_tr = (ident[:, :].bitcast(f32) if MM_DT == "f32r"
                                else ident[:, :])
                    for m in range(M):
                        gps = ppool.tile([128, GATE_COLS], f32, tag="gates")
                        # inject xg (clears + seeds accumulation per bank)
                        for (c0, c1) in CHUNKS:
                            nc.tensor.matmul(
                                gps[:, c0:c1],
                                lhsT=_mm(identg[:, :], G_DT),
                                rhs=_mm(xg[:, m, c0:c1], G_DT),
                                start=True,
                                stop=False,
                            )
                        # gates += [h | 1] @ waug
                        for k, (d0, d1) in enumerate(K_SPLITS):
                            lhs = hT[k][:, m * 128:(m + 1) * 128]
                            for (c0, c1) in CHUNKS:
                                nc.tensor.matmul(
                                    gps[:, c0:c1],
                                    lhsT=_mm(lhs, MM_DT),
                                    rhs=_mm(waug_sb[k][:, c0:c1], MM_DT),
                                    start=False,
                                    stop=(k == 2),
                                )
                        nc.scalar.activation(
                            ifo[:, m, :], gps[:, 0:900],
                            mybir.ActivationFunctionType.Sigmoid,
                        )
                        nc.scalar.activation(
                            gt[:, m, :], gps[:, 900:1200],
                            mybir.ActivationFunctionType.Tanh,
                        )
                        # per-m state update: c = f*c + i*g ; h = o*tanh(c)
                        nc.vector.tensor_tensor(
                            out=t1[:, m, :], in0=ifo[:, m, 300:600],
                            in1=c_sb[:, m, :], op=mybir.AluOpType.mult,
                        )
                        nc.vector.tensor_tensor(
                            out=c_sb[:, m, :], in0=ifo[:, m, 0:300],
                            in1=gt[:, m, :], op=mybir.AluOpType.mult,
                        )
                        nc.vector.tensor_tensor(
                            out=c_sb[:, m, :], in0=c_sb[:, m, :],
                            in1=t1[:, m, :], op=mybir.AluOpType.add,
                        )
                        nc.scalar.activation(
                            tch[:, m, :], c_sb[:, m, :],
                            mybir.ActivationFunctionType.Tanh,
                        )
                        nc.vector.tensor_tensor(
                            out=h_sb[:, m, :], in0=ifo[:, m, 600:900],
                            in1=tch[:, m, :], op=mybir.AluOpType.mult,
                        )
                        nc.vector.copy_predicated(
                            out=h_last[:, m, :],
                            mask=mask_sb[:, t, m:m + 1].to_broadcast([128, H]),
                            data=h_sb[:, m, :],
                        )
                        # h -> hT for the next step (per-m transpose + drain)
                        trp = ppool.tile([128, 384], f32 if MM_DT == "f32r"
                                         else mm_np_dt, tag="tr")
                        for k, (d0, d1) in enumerate(H_SPLITS):
                            dk = d1 - d0
                            nc.tensor.transpose(
                                out=trp[0:dk, k * 128:k * 128 + 128],
                                in_=h_sb[:, m, d0:d1],
                                identity=ident_tr,
                            )
                        for k, (d0, d1) in enumerate(H_SPLITS):
                            dk = d1 - d0
                            dst = hT[k][0:dk, m * 128:(m + 1) * 128]
                            srcp = trp[0:dk, k * 128:k * 128 + 128]
                            if (m + k) % 2 == 0:
                                nc.vector.tensor_copy(dst, srcp)
                            else:
                                nc.scalar.copy(dst, srcp)

                # ---------- phase 3: logits ----------
                trps = []
                for k in range(3):
                    trp = ppool.tile([128, 512], f32 if MM_DT == "f32r" else mm_np_dt, tag="tr")
                    trps.append(trp)
                for m in range(M):
                    for k, (d0, d1) in enumerate(H_SPLITS):
                        dk = d1 - d0
                        nc.tensor.transpose(
                            out=_mm(trps[k][0:dk, m * 128:(m + 1) * 128], MM_DT),
                            in_=_mm(h_last[:, m, d0:d1], MM_DT),
                            identity=ident[:, :].bitcast(f32) if MM_DT == "f32r" else ident[:, :],
                        )
                for k, (d0, d1) in enumerate(H_SPLITS):
                    dk = d1 - d0
                    nc.vector.tensor_copy(hlT[k][0:dk, :], trps[k][0:dk, :])

                lsb = wpool.tile([128, M, 2], f32, tag="lsb")
                for m in range(M):
                    lp = ppool.tile([128, 2], f32, tag="tr")
                    for k, (d0, d1) in enumerate(K_SPLITS):
                        nc.tensor.matmul(
                            lp[:, :],
                            lhsT=_mm(hlT[k][:, m * 128:(m + 1) * 128], MM_DT),
                            rhs=_mm(wc_sb[k][:, :], MM_DT),
                            start=(k == 0),
                            stop=(k == 2),
                        )
                    nc.vector.tensor_copy(lsb[:, m, :], lp[:, :])
                nc.sync.dma_start(
                    out=out_d[:, :].rearrange("(m p) c -> p m c", p=128), in_=lsb[:, :, :]
                )

    nc.compile()
    return nc


_NC_CACHE = {}
LAST_RESULT = None


def _host_prep(inputs):
    mm_np = np.float32
    g_np = np.float32
    if MM_DT == "bf16" or G_DT == "bf16":
        import ml_dtypes
        if MM_DT == "bf16":
            mm_np = ml_dtypes.bfloat16
        if G_DT == "bf16":
            g_np = ml_dtypes.bfloat16
    del g_np  # G dtype handled on device (psum -> gsb copy casts)

    cap = np.asarray(inputs["cap"]).astype(np.int32)
    cap_len = np.asarray(inputs["cap_len"]).astype(np.int32)
    embed_w = np.asarray(inputs["embed_w"], dtype=np.float32)
    W_ih = np.asarray(inputs["W_ih"], dtype=np.float32)
    W_hh = np.asarray(inputs["W_hh"], dtype=np.float32)
    b = (np.asarray(inputs["b_ih"], dtype=np.float32)
         + np.asarray(inputs["b_hh"], dtype=np.float32))
    cls_v = np.asarray(inputs["cls_v"], dtype=np.float32)
    cls_g = np.asarray(inputs["cls_g"], dtype=np.float32)
    cls_b = np.asarray(inputs["cls_b"], dtype=np.float32)

    perm = np.concatenate([np.arange(0, 300), np.arange(300, 600),
                           np.arange(900, 1200), np.arange(600, 900)])  # i f o g
    wih_t = np.zeros((E, GATE_COLS), np.float32)
    wih_t[:, :1200] = W_ih[perm].T
    waug = np.zeros((H + 1, GATE_COLS), np.float32)
    waug[:H, :1200] = W_hh[perm].T
    waug[H, :1200] = b[perm]
    Wc = cls_g * cls_v / np.linalg.norm(cls_v, axis=1, keepdims=True)  # [2, 300]
    wc = np.zeros((H + 1, 2), np.float32)
    wc[:H] = Wc.T
    wc[H] = cls_b
    embT = np.ascontiguousarray(embed_w.T)  # [300, 10000]

    eye = np.eye(128, dtype=np.float32)
    hT_init = np.zeros((H + 1, BC), np.float32)
    hT_init[H] = 1.0
    shared = {
        "ident": eye.astype(mm_np),
        "identg": eye.astype(mm_np if G_DT == MM_DT else (
            __import__("ml_dtypes").bfloat16 if G_DT == "bf16" else np.float32)),
        "hT_init": hT_init.astype(mm_np),
        "embT": embT.astype(mm_np),
        "wih": wih_t.astype(mm_np),
        "waug": waug.astype(mm_np),
        "wc": wc.astype(mm_np),
    }
    in_maps = []
    for core in range(NCORES):
        capc = cap[core * BC:(core + 1) * BC]          # [512, 32]
        lenc = cap_len[core * BC:(core + 1) * BC]      # [512]
        idx = np.ascontiguousarray(
            capc.reshape(M, 128, T).transpose(1, 2, 0)).astype(np.int32)  # [128,T,M]
        lm = lenc.reshape(M, 128).T                    # [128, M]
        mask = (lm[:, None, :] - 1 == np.arange(T)[None, :, None]).astype(np.uint8)
        in_maps.append(dict(shared, idx=idx, mask=np.ascontiguousarray(mask)))
    return in_maps


def kernel(**inputs) -> np.ndarray:
    global LAST_RESULT
    key = (MM_DT, G_DT)
    if key not in _NC_CACHE:
        _NC_CACHE[key] = build_bass()
    nc = _NC_CACHE[key]
    in_maps = _host_prep(inputs)
    trace = bool(int(os.environ.get("KERNEL_TRACE", "0")))
    res = run_bass_kernel_spmd(nc, in_maps, core_ids=list(range(NCORES)), trace=trace)
    LAST_RESULT = res
    out = np.concatenate([r["out"] for r in res.results], axis=0)
    return out.astype(np.float32)



# revision 39
# speedup vs baseline: 1.8091x; 1.8091x over previous
"""Trainium2 Bass kernel for nn_Discriminator_lstm (B=4096, T=32, E=H=300, VOCAB=10000).

Strategy (data-parallel over batch, 8 cores x 512 rows, all-bf16):
  Host: rows are sorted by cap_len descending and dealt round-robin to the
  8 cores, so each core's 4 m-tiles (128 rows) die off as sequences end.
  The compiled program is specialized to the per-step active-tile schedule
  (compile time is not part of HW exec time); dead tiles are skipped,
  cutting ~37% of all per-step engine work.

  Per step t, per active m-tile, a fused state vector eh = [e_t | h_{t-1} | 1]
  (640 cols, bf16, double-buffered by t parity) is:
    - filled by an indirect-DMA gather of embedding rows (e_t, prefetched
      two steps ahead) and by the previous step's h write,
    - transposed in one xbar DMA (no PE/PSUM) into ehT [128, 5, 128],
    - multiplied against stacked weights Wcomb = [W_ih; W_hh; b] in 5
      k-passes x 3 PSUM chunks (1200 gate cols, order i f o | g),
  then ACT applies sigmoid/tanh out of PSUM into bf16, DVE updates c and h
  (bf16 2x/4x modes), h_last is captured by predicated copy on the tiles
  where sequences end, and the logits use the same xbar-transpose trick on
  h_last at the end.
"""

import os
import sys

import numpy as np

for _p in ("/opt/trn_rl_repo", "/root/.axon_site/_ro/trn_rl_repo"):
    if os.path.isdir(_p) and _p not in sys.path:
        sys.path.insert(0, _p)

import concourse.bass as bass
import concourse.bacc as bacc
import concourse.mybir as mybir
import concourse.tile as tile
from concourse.bass_utils import run_bass_kernel_spmd

f32 = mybir.dt.float32
bf16 = mybir.dt.bfloat16
i32 = mybir.dt.int32
u8 = mybir.dt.uint8

B, T, V, E, H = 4096, 32, 10000, 300, 300
NCORES = 8
BC = B // NCORES          # 512 batch rows per core
M = BC // 128             # 4 m-tiles
GCOLS = 1200              # gates: [i f o | g], sigmoid on 0:900, tanh on 900:1200
CHUNKS = [(0, 512), (512, 1024), (1024, 1200)]
EH = 640                  # [e(300) | h(300) | 1 | pad] -> 5 k-splits of 128
KS = [(0, 128), (128, 256), (256, 384), (384, 512), (512, 601)]
HL = 384                  # h_last padded: [h(300) | 1 | 0-pad]

Sig = mybir.ActivationFunctionType.Sigmoid
Tanh = mybir.ActivationFunctionType.Tanh
Mult = mybir.AluOpType.mult
Add = mybir.AluOpType.add

# PE warmth filler: scratch matmuls emitted after low-occupancy steps so the
# PE p-state ramp (0.65/1.2/2.4 GHz) survives the serial recurrence chain.
WARM_COLS = 512
WARM_N = {1: int(os.environ.get("KERNEL_WARM1", "14")),
          2: int(os.environ.get("KERNEL_WARM2", "7")),
          3: int(os.environ.get("KERNEL_WARM3", "0")),
          4: 0}


STEP_TIMES = []  # (step, engine, sim_time) records when KERNEL_PROBE=1


def build_bass(amt, cap_tiles):
    """amt[t] = number of active m-tiles at step t (non-increasing).
    cap_tiles[t] = tuple of m-tiles that may capture h_last at step t."""
    probe = bool(int(os.environ.get("KERNEL_PROBE", "0")))
    if probe:
        from concourse.bass_interp import add_callback2
    nc = bacc.Bacc("TRN2", target_bir_lowering=False, debug=False,
                   num_devices=NCORES)

    embw_d = nc.dram_tensor("embw", [V, E], bf16, kind="ExternalInput")
    ident_d = nc.dram_tensor("ident", [128, 128], bf16, kind="ExternalInput")
    wcomb_d = nc.dram_tensor("wcomb", [601, GCOLS], bf16, kind="ExternalInput")
    wc_d = nc.dram_tensor("wc", [HL, 2], bf16, kind="ExternalInput")
    idx_d = nc.dram_tensor("idx", [128, T, M], i32, kind="ExternalInput")
    mask_d = nc.dram_tensor("mask", [128, T, M], u8, kind="ExternalInput")
    out_d = nc.dram_tensor("out", [BC, 2], f32, kind="ExternalOutput")

    with tile.TileContext(nc, num_cores=NCORES) as tc:
        with (
            tc.tile_pool(name="const", bufs=1) as cpool,
            tc.tile_pool(name="state", bufs=1) as spool,
            tc.tile_pool(name="eht", bufs=2) as tpool,
            tc.tile_pool(name="act", bufs=2) as apool,
            tc.tile_pool(name="psumA", bufs=2, space="PSUM") as ppoolA,
            tc.tile_pool(name="psumT", bufs=2, space="PSUM") as ppoolT,
        ):
            # ---------- constants ----------
            ident = cpool.tile([128, 128], bf16, tag="ident")
            nc.sync.dma_start(out=ident[:, :], in_=ident_d[:, :])
            wcomb_sb = []
            for k, (d0, d1) in enumerate(KS):
                t_ = cpool.tile([d1 - d0, GCOLS], bf16, tag=f"wcomb{k}")
                nc.sync.dma_start(out=t_[:, :], in_=wcomb_d[d0:d1, :])
                wcomb_sb.append(t_)
            wc_sb = []
            for k in range(3):
                t_ = cpool.tile([128, 2], bf16, tag=f"wc{k}")
                nc.sync.dma_start(out=t_[:, :], in_=wc_d[k * 128:(k + 1) * 128, :])
                wc_sb.append(t_)
            idx_sb = cpool.tile([128, T, M], i32, tag="idx")
            nc.sync.dma_start(out=idx_sb[:, :, :], in_=idx_d[:, :, :])
            mask_sb = cpool.tile([128, T, M], u8, tag="mask")
            nc.sync.dma_start(out=mask_sb[:, :, :], in_=mask_d[:, :, :])

            ablate = os.environ.get("KERNEL_ABLATE", "none")
            # ---------- state ----------
            # eh[p, m, buf, :] = [e_t(300) | h_{t-1}(300) | 1 | 0-pad(39)]
            eh = spool.tile([128, M, 2, EH], bf16, tag="eh")
            nc.gpsimd.memset(eh[:, :, :, :], 0.0)
            nc.gpsimd.memset(eh[:, :, :, 600:601], 1.0)
            c_sb = spool.tile([128, M, H], bf16, tag="c")
            nc.gpsimd.memset(c_sb[:, :, :], 0.0)
            h_last = spool.tile([128, M, HL], bf16, tag="hlast")
            nc.gpsimd.memset(h_last[:, :, :], 0.0)
            nc.gpsimd.memset(h_last[:, :, 300:301], 1.0)
            eh0 = None
            if ablate == "notr":
                eh0 = spool.tile([128, M, EH], bf16, tag="eh0")
                nc.gpsimd.memset(eh0[:, :, :], 0.0)

            # ---------- prologue gathers (steps 0 and 1) ----------
            for t in range(2):
                if t < T and amt[t] > 0:
                    for m in range(amt[t]):
                        nc.gpsimd.indirect_dma_start(
                            out=eh[:, m, t % 2, 0:E],
                            out_offset=None,
                            in_=embw_d[:, :],
                            in_offset=bass.IndirectOffsetOnAxis(
                                ap=idx_sb[:, t, m:m + 1], axis=0),
                        )

            # ---------- recurrence ----------
            for t in range(T):
                a = amt[t]
                if a == 0:
                    break
                buf, nbuf = t % 2, (t + 1) % 2

                def finish(m, t=t, buf=buf, nbuf=nbuf):
                    """Deferred tail of tile m's step: tanh(c), h write, capture.
                    Emitted one m-slot late so ACT's gate activations (which
                    free the gates PSUM for PE) are never queued behind it."""
                    tch = apool.tile([128, H], bf16, tag=f"tch{m % 2}")
                    nc.scalar.activation(tch[:, :], c_sb[:, m, :], Tanh)
                    ifo = ifo_tiles[m]
                    nc.vector.tensor_tensor(
                        out=eh[:, m, nbuf, 300:600], in0=ifo[:, 600:900],
                        in1=tch[:, :], op=Mult)
                    if m in cap_tiles[t]:
                        nc.vector.copy_predicated(
                            out=h_last[:, m, 0:300],
                            mask=mask_sb[:, t, m:m + 1].to_broadcast([128, H]),
                            data=eh[:, m, nbuf, 300:600],
                        )

                ifo_tiles = {}
                for m in range(a):
                    # eh -> ehT on PE (chain latency ~1.2us vs ~3.4us for the
                    # xbar DMA path: no HWDGE/DGE delay, no 900ns DMA sem)
                    ehT = tpool.tile([128, 5, 128], bf16, tag=f"ehT{m}")
                    trp = ppoolT.tile([128, 5 * 128], bf16, tag="trp")
                    for j in range(5):
                        nc.tensor.transpose(
                            out=trp[:, j * 128:(j + 1) * 128],
                            in_=eh[:, m, buf, j * 128:(j + 1) * 128],
                            identity=ident[:, :],
                        )
                    for j in range(5):
                        if j % 2 == 0:
                            nc.vector.tensor_copy(
                                ehT[:, j, :], trp[:, j * 128:(j + 1) * 128])
                        else:
                            nc.scalar.copy(
                                ehT[:, j, :], trp[:, j * 128:(j + 1) * 128])
                    # prefetch the gather for step t+2 into this buffer
                    if t + 2 < T and m < amt[t + 2]:
                        nc.gpsimd.indirect_dma_start(
                            out=eh[:, m, buf, 0:E],
                            out_offset=None,
                            in_=embw_d[:, :],
                            in_offset=bass.IndirectOffsetOnAxis(
                                ap=idx_sb[:, t + 2, m:m + 1], axis=0),
                        )
                    gps = ppoolA.tile([128, GCOLS], f32, tag="gates")
                    if probe:
                        add_callback2(
                            nc.tensor,
                            (lambda s, i, t=t, m=m: STEP_TIMES.append(
                                (t, f"PEs{m}", s.time, s.pe_busy_start))),
                            ins=[ehT[0:1, 0, 0:1]])
                    for k, (d0, d1) in enumerate(KS):
                        kr = d1 - d0
                        for (c0, c1) in CHUNKS:
                            nc.tensor.matmul(
                                gps[:, c0:c1], lhsT=ehT[0:kr, k, :],
                                rhs=wcomb_sb[k][:, c0:c1],
                                start=(k == 0), stop=(k == 4))
                    if probe:
                        add_callback2(
                            nc.tensor,
                            (lambda s, i, t=t, m=m: STEP_TIMES.append(
                                (t, f"PE{m}", s.time, s.pe_busy_start))),
                            ins=[gps[:, 0:1]])
                    ifo = apool.tile([128, 900], bf16, tag=f"ifo{m % 3}")
                    ifo_tiles[m] = ifo
                    gt = apool.tile([128, H], bf16, tag=f"gt{m % 2}")
                    if a <= 2:
                        # f first: unblocks t1 = f*c on DVE after ~335ns
                        nc.scalar.activation(ifo[:, 300:600], gps[:, 300:600], Sig)
                        nc.scalar.activation(ifo[:, 0:300], gps[:, 0:300], Sig)
                        nc.scalar.activation(gt[:, :], gps[:, 900:1200], Tanh)
                        nc.scalar.activation(ifo[:, 600:900], gps[:, 600:900], Sig)
                    else:
                        nc.scalar.activation(ifo[:, :], gps[:, 0:900], Sig)
                        nc.scalar.activation(gt[:, :], gps[:, 900:1200], Tanh)
                    t1 = apool.tile([128, H], bf16, tag=f"t1{m % 2}")
                    nc.vector.tensor_tensor(
                        out=t1[:, :], in0=ifo[:, 300:600], in1=c_sb[:, m, :],
                        op=Mult)
                    nc.vector.tensor_tensor(
                        out=c_sb[:, m, :], in0=ifo[:, 0:300], in1=gt[:, :],
                        op=Mult)
                    nc.vector.tensor_tensor(
                        out=c_sb[:, m, :], in0=c_sb[:, m, :], in1=t1[:, :],
                        op=Add)
                    if m >= 1:
                        finish(m - 1)
                finish(a - 1)
                if probe:
                    add_callback2(
                        nc.vector,
                        (lambda s, i, t=t: STEP_TIMES.append(
                            (t, "h0", s.time, s.pe_busy_start))),
                        ins=[eh[:, 0, nbuf, 300:600]])


            # ---------- logits ----------
            lsb = spool.tile([128, M, 2], f32, tag="lsb")
            for m in range(M):
                hlT = tpool.tile([128, 3, 128], bf16, tag="hlT")
                nc.sync.dma_start_transpose(hlT[:, :, :], h_last[:, m, :])
                lp = ppoolA.tile([128, GCOLS], f32, tag="gates")
                for k in range(3):
                    nc.tensor.matmul(
                        lp[:, 0:2],
                        lhsT=hlT[:, k, :],
                        rhs=wc_sb[k][:, :],
                        start=(k == 0),
                        stop=(k == 2),
                    )
                nc.vector.tensor_copy(lsb[:, m, :], lp[:, 0:2])
            nc.sync.dma_start(
                out=out_d[:, :].rearrange("(m p) c -> p m c", p=128),
                in_=lsb[:, :, :])

    nc.compile()
    return nc


_NC_CACHE = {}
LAST_RESULT = None
LAST_NC = None
LAST_IN_MAPS = None
LAST_ROWS = None


def _schedule(cap_len_by_core):
    """amt[t] (max over cores) and capture-tile sets (union over cores)."""
    amt = []
    cap_tiles = []
    for t in range(T):
        a = 0
        caps = set()
        for lens in cap_len_by_core:
            n = int((lens > t).sum())
            a = max(a, -(-n // 128))
            pos = np.nonzero(lens == t + 1)[0]
            if len(pos):
                caps.update(range(int(pos[0]) // 128, int(pos[-1]) // 128 + 1))
        amt.append(a)
        cap_tiles.append(tuple(sorted(caps)))
    return amt, cap_tiles


def _host_prep(inputs):
    import ml_dtypes
    bf = ml_dtypes.bfloat16

    cap = np.asarray(inputs["cap"]).astype(np.int32)
    cap_len = np.asarray(inputs["cap_len"]).astype(np.int64)
    embed_w = np.asarray(inputs["embed_w"], dtype=np.float32)
    W_ih = np.asarray(inputs["W_ih"], dtype=np.float32)
    W_hh = np.asarray(inputs["W_hh"], dtype=np.float32)
    b = (np.asarray(inputs["b_ih"], dtype=np.float32)
         + np.asarray(inputs["b_hh"], dtype=np.float32))
    cls_v = np.asarray(inputs["cls_v"], dtype=np.float32)
    cls_g = np.asarray(inputs["cls_g"], dtype=np.float32)
    cls_b = np.asarray(inputs["cls_b"], dtype=np.float32)

    # gate order [i f o g]: sigmoid block 0:900, tanh block 900:1200
    perm = np.concatenate([np.arange(0, 300), np.arange(300, 600),
                           np.arange(900, 1200), np.arange(600, 900)])
    wcomb = np.zeros((601, GCOLS), np.float32)
    wcomb[0:300] = W_ih[perm].T
    wcomb[300:600] = W_hh[perm].T
    wcomb[600] = b[perm]
    Wc = cls_g * cls_v / np.linalg.norm(cls_v, axis=1, keepdims=True)  # [2,300]
    wc = np.zeros((HL, 2), np.float32)
    wc[0:300] = Wc.T
    wc[300] = cls_b

    # length-descending sort, dealt round-robin across cores
    order = np.argsort(-cap_len, kind="stable")
    rows = [order[c::NCORES] for c in range(NCORES)]   # each [512], desc lens
    lens_by_core = [cap_len[r] for r in rows]
    amt, cap_tiles = _schedule(lens_by_core)

    shared = {
        "embw": embed_w.astype(bf),
        "wcomb": wcomb.astype(bf),
        "wc": wc.astype(bf),
        "ident": np.eye(128, dtype=np.float32).astype(bf),
    }
    in_maps = []
    for c in range(NCORES):
        capc = cap[rows[c]]                            # [512, 32]
        lenc = cap_len[rows[c]]                        # [512]
        idx = np.ascontiguousarray(
            capc.reshape(M, 128, T).transpose(1, 2, 0)).astype(np.int32)
        lm = lenc.reshape(M, 128).T                    # [128, M]
        mask = (lm[:, None, :] - 1 == np.arange(T)[None, :, None]).astype(np.uint8)
        in_maps.append(dict(shared, idx=idx, mask=np.ascontiguousarray(mask)))
    return amt, cap_tiles, in_maps, rows


def kernel(**inputs) -> np.ndarray:
    global LAST_RESULT, LAST_NC, LAST_IN_MAPS, LAST_ROWS
    amt, cap_tiles, in_maps, rows = _host_prep(inputs)
    key = (tuple(amt), tuple(cap_tiles))
    if key not in _NC_CACHE:
        _NC_CACHE[key] = build_bass(amt, cap_tiles)
    nc = _NC_CACHE[key]
    LAST_NC, LAST_IN_MAPS, LAST_ROWS = nc, in_maps, rows
    trace = bool(int(os.environ.get("KERNEL_TRACE", "0")))
    res = run_bass_kernel_spmd(nc, in_maps, core_ids=list(range(NCORES)),
                               trace=trace)
    LAST_RESULT = res
    out = np.empty((B, 2), np.float32)
    for c in range(NCORES):
        out[rows[c]] = res.results[c]["out"].astype(np.float32)
    return out


# revision 48
# speedup vs baseline: 1.9414x; 1.0731x over previous
"""Trainium2 Bass kernel for nn_Discriminator_lstm (B=4096, T=32, E=H=300, VOCAB=10000).

Strategy (data-parallel over batch, 8 cores x 512 rows, all-bf16):
  Host: rows are sorted by cap_len descending and dealt round-robin to the
  8 cores, so each core's 4 m-tiles (128 rows) die off as sequences end.
  The compiled program is specialized to the per-step active-tile schedule
  (compile time is not part of HW exec time); dead tiles are skipped,
  cutting ~37% of all per-step engine work.

  Per step t, per active m-tile, a fused state vector eh = [e_t | h_{t-1} | 1]
  (640 cols, bf16, double-buffered by t parity) is:
    - filled by an indirect-DMA gather of embedding rows (e_t, prefetched
      two steps ahead) and by the previous step's h write,
    - transposed on PE (5x [128,128] + DVE drains) into ehT [128, 5, 128];
      the e-only k-slices (j0/j1) and gate passes k0/k1 run a step EARLY,
      off the h -> gates serial chain,
    - multiplied against stacked weights Wcomb = [W_ih; W_hh; b] in 5
      k-passes x 3 PSUM chunks (1200 gate cols, order i f o | g),
  then ACT applies sigmoid/tanh out of PSUM into bf16 (i,f first in the
  latency-bound tail), DVE updates c and h, h_last is captured one step
  late (arithmetic on Pool in the tail) on tiles where sequences end, and
  the logits reuse the PE-transpose trick on h_last.
"""

import os
import sys

import numpy as np

for _p in ("/opt/trn_rl_repo", "/root/.axon_site/_ro/trn_rl_repo"):
    if os.path.isdir(_p) and _p not in sys.path:
        sys.path.insert(0, _p)

import concourse.bass as bass
import concourse.bacc as bacc
import concourse.mybir as mybir
import concourse.tile as tile
from concourse.bass_utils import run_bass_kernel_spmd

f32 = mybir.dt.float32
bf16 = mybir.dt.bfloat16
i32 = mybir.dt.int32
u8 = mybir.dt.uint8

B, T, V, E, H = 4096, 32, 10000, 300, 300
NCORES = 8
BC = B // NCORES          # 512 batch rows per core
M = BC // 128             # 4 m-tiles
GCOLS = 1200              # gates: [i f o | g], sigmoid on 0:900, tanh on 900:1200
CHUNKS = [(0, 512), (512, 1024), (1024, 1200)]
EH = 640                  # [e(300) | h(300) | 1 | pad] -> 5 k-splits of 128
KS = [(0, 128), (128, 256), (256, 384), (384, 512), (512, 601)]
HL = 384                  # h_last padded: [h(300) | 1 | 0-pad]

Sig = mybir.ActivationFunctionType.Sigmoid
Tanh = mybir.ActivationFunctionType.Tanh
Mult = mybir.AluOpType.mult
Add = mybir.AluOpType.add

STEP_TIMES = []  # (step, engine, sim_time) records when KERNEL_PROBE=1


def build_bass(amt, cap_tiles):
    """amt[t] = number of active m-tiles at step t (non-increasing).
    cap_tiles[t] = tuple of m-tiles that may capture h_last at step t."""
    probe = bool(int(os.environ.get("KERNEL_PROBE", "0")))
    if probe:
        from concourse.bass_interp import add_callback2
    nc = bacc.Bacc("TRN2", target_bir_lowering=False, debug=False,
                   num_devices=NCORES)

    embw_d = nc.dram_tensor("embw", [V, E], bf16, kind="ExternalInput")
    ident_d = nc.dram_tensor("ident", [128, 128], bf16, kind="ExternalInput")
    wcomb_d = nc.dram_tensor("wcomb", [601, GCOLS], bf16, kind="ExternalInput")
    wc_d = nc.dram_tensor("wc", [HL, 2], bf16, kind="ExternalInput")
    idx_d = nc.dram_tensor("idx", [128, T, M], i32, kind="ExternalInput")
    mask_d = nc.dram_tensor("mask", [128, T, M], u8, kind="ExternalInput")
    maskf_d = nc.dram_tensor("maskf", [128, T, M], bf16, kind="ExternalInput")
    out_d = nc.dram_tensor("out", [BC, 2], f32, kind="ExternalOutput")

    with tile.TileContext(nc, num_cores=NCORES) as tc:
        with (
            tc.tile_pool(name="const", bufs=1) as cpool,
            tc.tile_pool(name="state", bufs=1) as spool,
            tc.tile_pool(name="eht", bufs=3) as tpool,
            tc.tile_pool(name="act", bufs=3) as apool,
            tc.tile_pool(name="psumA", bufs=2, space="PSUM") as ppoolA,
            tc.tile_pool(name="psumT", bufs=2, space="PSUM") as ppoolT,
        ):
            # ---------- constants ----------
            # idx first: the prologue gathers (and everything downstream)
            # wait on it. Loads alternate SP/ACT HWDGE queues.
            idx_sb = cpool.tile([128, T, M], i32, tag="idx")
            nc.sync.dma_start(out=idx_sb[:, :, :], in_=idx_d[:, :, :])
            ident = cpool.tile([128, 128], bf16, tag="ident")
            nc.scalar.dma_start(out=ident[:, :], in_=ident_d[:, :])
            wcomb_sb = []
            for k, (d0, d1) in enumerate(KS):
                t_ = cpool.tile([d1 - d0, GCOLS], bf16, tag=f"wcomb{k}")
                eng = nc.sync if k % 2 == 0 else nc.scalar
                eng.dma_start(out=t_[:, :], in_=wcomb_d[d0:d1, :])
                wcomb_sb.append(t_)
            wc_sb = []
            for k in range(3):
                t_ = cpool.tile([128, 2], bf16, tag=f"wc{k}")
                eng = nc.sync if k % 2 == 0 else nc.scalar
                eng.dma_start(out=t_[:, :], in_=wc_d[k * 128:(k + 1) * 128, :])
                wc_sb.append(t_)
            mask_sb = cpool.tile([128, T, M], u8, tag="mask")
            nc.scalar.dma_start(out=mask_sb[:, :, :], in_=mask_d[:, :, :])
            maskf_sb = cpool.tile([128, T, M], bf16, tag="maskf")
            nc.scalar.dma_start(out=maskf_sb[:, :, :], in_=maskf_d[:, :, :])

            # ---------- state ----------
            # eh[p, m, buf, :] = [e_t(300) | h_{t-1}(300) | 1 | 0-pad(39)]
            eh = spool.tile([128, M, 2, EH], bf16, tag="eh")
            c_sb = spool.tile([128, M, H], bf16, tag="c")
            h_last = spool.tile([128, M, HL], bf16, tag="hlast")

            # ---------- prologue: gathers first (Pool is in-order), then
            # memsets for the h/ones/pad regions on DVE, off the gather path
            for t in range(2):
                if t < T and amt[t] > 0:
                    for m in range(amt[t]):
                        nc.gpsimd.indirect_dma_start(
                            out=eh[:, m, t % 2, 0:E],
                            out_offset=None,
                            in_=embw_d[:, :],
                            in_offset=bass.IndirectOffsetOnAxis(
                                ap=idx_sb[:, t, m:m + 1], axis=0),
                        )
            nc.vector.memset(eh[:, :, :, 300:600], 0.0)
            nc.vector.memset(eh[:, :, :, 600:601], 1.0)
            nc.vector.memset(eh[:, :, :, 601:640], 0.0)
            nc.vector.memset(c_sb[:, :, :], 0.0)
            nc.vector.memset(h_last[:, :, :], 0.0)
            nc.vector.memset(h_last[:, :, 300:301], 1.0)

            # ---------- recurrence ----------
            # Early/late split: transposes j0/j1 and matmul k-passes 0/1 touch
            # only the gathered embedding columns (dims 0..255), so they run a
            # step ahead, off the h -> gates critical chain. j2..j4 / k2..k4
            # need h_{t-1} (dims 256..601 overlap h at 300..600).
            ehTD, trpD, gpsD = {}, {}, {}

            def early(t, m):
                """Allocate next-step tiles for (t, m); emit e-only transposes,
                drains, and k0/k1 gate passes. Reads eh[:, m, t%2, 0:256]."""
                b = t % 2
                trp = ppoolT.tile([128, 5 * 128], bf16, tag="trp")
                ehT = tpool.tile([128, 5, 128], bf16, tag=f"ehT{m}")
                ehTD[m], trpD[m] = ehT, trp
                for j in (0, 1):
                    nc.tensor.transpose(
                        out=trp[:, j * 128:(j + 1) * 128],
                        in_=eh[:, m, b, j * 128:(j + 1) * 128],
                        identity=ident[:, :])
                nc.vector.tensor_copy(ehT[:, 0, :], trp[:, 0:128])
                nc.vector.tensor_copy(ehT[:, 1, :], trp[:, 128:256])
                gps = ppoolA.tile([128, GCOLS], f32, tag="gates")
                gpsD[m] = gps
                for k in (0, 1):
                    for (c0, c1) in CHUNKS:
                        nc.tensor.matmul(
                            gps[:, c0:c1], lhsT=ehT[:, k, :],
                            rhs=wcomb_sb[k][:, c0:c1],
                            start=(k == 0), stop=False)

            for m in range(amt[0]):
                early(0, m)

            pending_cap = []
            for t in range(T):
                a = amt[t]
                if a == 0:
                    break
                buf, nbuf = t % 2, (t + 1) % 2

                def finish(m, t=t, buf=buf, nbuf=nbuf):
                    """Deferred tail of tile m's step: tanh(c), h write.
                    Emitted one m-slot late so ACT's gate activations (which
                    free the gates PSUM for PE) are never queued behind it."""
                    tch = apool.tile([128, H], bf16, tag=f"tch{m % 2}")
                    nc.scalar.activation(tch[:, :], c_sb[:, m, :], Tanh)
                    ifo = ifo_tiles[m]
                    nc.vector.tensor_tensor(
                        out=eh[:, m, nbuf, 300:600], in0=ifo[:, 600:900],
                        in1=tch[:, :], op=Mult)
                    if m in cap_tiles[t]:
                        pending_cap.append((t, m, nbuf))

                def flush_caps_prev(a=a):
                    # h_t stays valid in its parity buffer for two steps, so
                    # the capture can run a step late, off the DVE chain.
                    # In the latency-bound tail it runs as arithmetic on the
                    # otherwise-idle Pool engine instead of DVE.
                    while pending_cap:
                        ct, cm, cb = pending_cap.pop(0)
                        if a <= 2:
                            capd = apool.tile([128, H], bf16, tag="capd")
                            nc.gpsimd.tensor_tensor(
                                out=capd[:, :], in0=eh[:, cm, cb, 300:600],
                                in1=h_last[:, cm, 0:300],
                                op=mybir.AluOpType.subtract)
                            nc.gpsimd.tensor_tensor(
                                out=capd[:, :], in0=capd[:, :],
                                in1=maskf_sb[:, ct, cm:cm + 1].to_broadcast(
                                    [128, H]),
                                op=Mult)
                            nc.gpsimd.tensor_tensor(
                                out=h_last[:, cm, 0:300],
                                in0=h_last[:, cm, 0:300], in1=capd[:, :],
                                op=Add)
                        else:
                            nc.vector.copy_predicated(
                                out=h_last[:, cm, 0:300],
                                mask=mask_sb[:, ct, cm:cm + 1].to_broadcast(
                                    [128, H]),
                                data=eh[:, cm, cb, 300:600],
                            )

                ifo_tiles = {}
                for m in range(a):
                    ehT, trp, gps = ehTD[m], trpD[m], gpsD[m]
                    if m == a - 1:
                        flush_caps_prev()
                    # late transposes (need h_{t-1}) + drains + k2..k4
                    for j in (2, 3, 4):
                        nc.tensor.transpose(
                            out=trp[:, j * 128:(j + 1) * 128],
                            in_=eh[:, m, buf, j * 128:(j + 1) * 128],
                            identity=ident[:, :])
                    nc.vector.tensor_copy(ehT[:, 2, :], trp[:, 256:384])
                    nc.vector.tensor_copy(ehT[:, 3, :], trp[:, 384:512])
                    nc.vector.tensor_copy(ehT[:, 4, :], trp[:, 512:640])
                    for k in (2, 3, 4):
                        d0, d1 = KS[k]
                        kr = d1 - d0
                        for (c0, c1) in CHUNKS:
                            nc.tensor.matmul(
                                gps[:, c0:c1], lhsT=ehT[0:kr, k, :],
                                rhs=wcomb_sb[k][:, c0:c1],
                                start=False, stop=(k == 4))
                    if probe:
                        add_callback2(
                            nc.tensor,
                            (lambda s, i, t=t, m=m: STEP_TIMES.append(
                                (t, f"PE{m}", s.time, s.pe_busy_start))),
                            ins=[gps[:, 0:1]])
                    # prefetch the gather for step t+2 into this buffer
                    if t + 2 < T and m < amt[t + 2]:
                        nc.gpsimd.indirect_dma_start(
                            out=eh[:, m, buf, 0:E],
                            out_offset=None,
                            in_=embw_d[:, :],
                            in_offset=bass.IndirectOffsetOnAxis(
                                ap=idx_sb[:, t + 2, m:m + 1], axis=0),
                        )
                    # next step's early work (e-part is already gathered)
                    if t + 1 < T and m < amt[t + 1]:
                        early(t + 1, m)
                    ifo = apool.tile([128, 900], bf16, tag=f"ifo{m % 3}")
                    ifo_tiles[m] = ifo
                    gt = apool.tile([128, H], bf16, tag=f"gt{m % 2}")
                    if a <= 2:
                        # i,f first (one call), then g: unblocks the DVE c
                        # chain earliest; o (only needed for h) goes last
                        nc.scalar.activation(ifo[:, 0:600], gps[:, 0:600], Sig)
                        nc.scalar.activation(gt[:, :], gps[:, 900:1200], Tanh)
                        nc.scalar.activation(ifo[:, 600:900], gps[:, 600:900], Sig)
                    else:
                        nc.scalar.activation(ifo[:, :], gps[:, 0:900], Sig)
                        nc.scalar.activation(gt[:, :], gps[:, 900:1200], Tanh)
                    t1 = apool.tile([128, H], bf16, tag=f"t1{m % 2}")
                    nc.vector.tensor_tensor(
                        out=t1[:, :], in0=ifo[:, 300:600], in1=c_sb[:, m, :],
                        op=Mult)
                    nc.vector.tensor_tensor(
                        out=c_sb[:, m, :], in0=ifo[:, 0:300], in1=gt[:, :],
                        op=Mult)
                    nc.vector.tensor_tensor(
                        out=c_sb[:, m, :], in0=c_sb[:, m, :], in1=t1[:, :],
                        op=Add)
                    if m >= 1:
                        finish(m - 1)
                finish(a - 1)
                if probe:
                    add_callback2(
                        nc.vector,
                        (lambda s, i, t=t: STEP_TIMES.append(
                            (t, "h0", s.time, s.pe_busy_start))),
                        ins=[eh[:, 0, nbuf, 300:600]])

            while pending_cap:
                ct, cm, cb = pending_cap.pop(0)
                nc.vector.copy_predicated(
                    out=h_last[:, cm, 0:300],
                    mask=mask_sb[:, ct, cm:cm + 1].to_broadcast([128, H]),
                    data=eh[:, cm, cb, 300:600],
                )

            # ---------- logits ----------
            lsb = spool.tile([128, M, 2], f32, tag="lsb")
            for m in range(M):
                hlT = tpool.tile([128, 3, 128], bf16, tag="hlT")
                trp3 = ppoolT.tile([128, 5 * 128], bf16, tag="trp")
                for j in range(3):
                    nc.tensor.transpose(
                        out=trp3[:, j * 128:(j + 1) * 128],
                        in_=h_last[:, m, j * 128:(j + 1) * 128],
                        identity=ident[:, :])
                for j in range(3):
                    nc.vector.tensor_copy(hlT[:, j, :],
                                          trp3[:, j * 128:(j + 1) * 128])
                lp = ppoolA.tile([128, GCOLS], f32, tag="gates")
                for k in range(3):
                    nc.tensor.matmul(
                        lp[:, 0:2],
                        lhsT=hlT[:, k, :],
                        rhs=wc_sb[k][:, :],
                        start=(k == 0),
                        stop=(k == 2),
                    )
                nc.vector.tensor_copy(lsb[:, m, :], lp[:, 0:2])
            nc.sync.dma_start(
                out=out_d[:, :].rearrange("(m p) c -> p m c", p=128),
                in_=lsb[:, :, :])

    nc.compile()
    return nc


_NC_CACHE = {}
LAST_RESULT = None
LAST_NC = None
LAST_IN_MAPS = None
LAST_ROWS = None


def _schedule(cap_len_by_core):
    """amt[t] (max over cores) and capture-tile sets (union over cores)."""
    amt = []
    cap_tiles = []
    for t in range(T):
        a = 0
        caps = set()
        for lens in cap_len_by_core:
            n = int((lens > t).sum())
            a = max(a, -(-n // 128))
            pos = np.nonzero(lens == t + 1)[0]
            if len(pos):
                caps.update(range(int(pos[0]) // 128, int(pos[-1]) // 128 + 1))
        amt.append(a)
        cap_tiles.append(tuple(sorted(caps)))
    return amt, cap_tiles


def _host_prep(inputs):
    import ml_dtypes
    bf = ml_dtypes.bfloat16

    cap = np.asarray(inputs["cap"]).astype(np.int32)
    cap_len = np.asarray(inputs["cap_len"]).astype(np.int64)
    embed_w = np.asarray(inputs["embed_w"], dtype=np.float32)
    W_ih = np.asarray(inputs["W_ih"], dtype=np.float32)
    W_hh = np.asarray(inputs["W_hh"], dtype=np.float32)
    b = (np.asarray(inputs["b_ih"], dtype=np.float32)
         + np.asarray(inputs["b_hh"], dtype=np.float32))
    cls_v = np.asarray(inputs["cls_v"], dtype=np.float32)
    cls_g = np.asarray(inputs["cls_g"], dtype=np.float32)
    cls_b = np.asarray(inputs["cls_b"], dtype=np.float32)

    # gate order [i f o g]: sigmoid block 0:900, tanh block 900:1200
    perm = np.concatenate([np.arange(0, 300), np.arange(300, 600),
                           np.arange(900, 1200), np.arange(600, 900)])
    wcomb = np.zeros((601, GCOLS), np.float32)
    wcomb[0:300] = W_ih[perm].T
    wcomb[300:600] = W_hh[perm].T
    wcomb[600] = b[perm]
    Wc = cls_g * cls_v / np.linalg.norm(cls_v, axis=1, keepdims=True)  # [2,300]
    wc = np.zeros((HL, 2), np.float32)
    wc[0:300] = Wc.T
    wc[300] = cls_b

    # length-descending sort, dealt round-robin across cores
    order = np.argsort(-cap_len, kind="stable")
    rows = [order[c::NCORES] for c in range(NCORES)]   # each [512], desc lens
    lens_by_core = [cap_len[r] for r in rows]
    amt, cap_tiles = _schedule(lens_by_core)

    shared = {
        "embw": embed_w.astype(bf),
        "wcomb": wcomb.astype(bf),
        "wc": wc.astype(bf),
        "ident": np.eye(128, dtype=np.float32).astype(bf),
    }
    in_maps = []
    for c in range(NCORES):
        capc = cap[rows[c]]                            # [512, 32]
        lenc = cap_len[rows[c]]                        # [512]
        idx = np.ascontiguousarray(
            capc.reshape(M, 128, T).transpose(1, 2, 0)).astype(np.int32)
        lm = lenc.reshape(M, 128).T                    # [128, M]
        mask = (lm[:, None, :] - 1 == np.arange(T)[None, :, None]).astype(np.uint8)
        in_maps.append(dict(shared, idx=idx, mask=np.ascontiguousarray(mask),
                            maskf=np.ascontiguousarray(mask).astype(bf)))
    return amt, cap_tiles, in_maps, rows


def kernel(**inputs) -> np.ndarray:
    global LAST_RESULT, LAST_NC, LAST_IN_MAPS, LAST_ROWS
    amt, cap_tiles, in_maps, rows = _host_prep(inputs)
    key = (tuple(amt), tuple(cap_tiles))
    if key not in _NC_CACHE:
        _NC_CACHE[key] = build_bass(amt, cap_tiles)
    nc = _NC_CACHE[key]
    LAST_NC, LAST_IN_MAPS, LAST_ROWS = nc, in_maps, rows
    trace = bool(int(os.environ.get("KERNEL_TRACE", "0")))
    res = run_bass_kernel_spmd(nc, in_maps, core_ids=list(range(NCORES)),
                               trace=trace)
    LAST_RESULT = res
    out = np.empty((B, 2), np.float32)
    for c in range(NCORES):
        out[rows[c]] = res.results[c]["out"].astype(np.float32)
    return out


# revision 52
# speedup vs baseline: 1.9501x; 1.0045x over previous
"""Trainium2 Bass kernel for nn_Discriminator_lstm (B=4096, T=32, E=H=300, VOCAB=10000).

Strategy (data-parallel over batch, 8 cores x 512 rows, all-bf16):
  Host: rows are sorted by cap_len descending and dealt round-robin to the
  8 cores, so each core's 4 m-tiles (128 rows) die off as sequences end.
  The compiled program is specialized to the per-step active-tile schedule
  (compile time is not part of HW exec time); dead tiles are skipped,
  cutting ~37% of all per-step engine work.

  Per step t, per active m-tile, a fused state vector eh = [e_t | h_{t-1} | 1]
  (640 cols, bf16, double-buffered by t parity) is:
    - filled by an indirect-DMA gather of embedding rows (e_t, prefetched
      two steps ahead) and by the previous step's h write,
    - transposed on PE (5x [128,128] + DVE drains) into ehT [128, 5, 128];
      the e-only k-slices (j0/j1) and gate passes k0/k1 run a step EARLY,
      off the h -> gates serial chain,
    - multiplied against stacked weights Wcomb = [W_ih; W_hh; b] in 5
      k-passes x 3 PSUM chunks (1200 gate cols, order i f o | g),
  then ACT applies sigmoid/tanh out of PSUM into bf16 (i,f first in the
  latency-bound tail), DVE updates c and h, h_last is captured one step
  late (arithmetic on Pool in the tail) on tiles where sequences end, and
  the logits reuse the PE-transpose trick on h_last.
"""

import os
import sys

import numpy as np

for _p in ("/opt/trn_rl_repo", "/root/.axon_site/_ro/trn_rl_repo"):
    if os.path.isdir(_p) and _p not in sys.path:
        sys.path.insert(0, _p)

import concourse.bass as bass
import concourse.bacc as bacc
import concourse.mybir as mybir
import concourse.tile as tile
from concourse.bass_utils import run_bass_kernel_spmd

f32 = mybir.dt.float32
bf16 = mybir.dt.bfloat16
i32 = mybir.dt.int32
u8 = mybir.dt.uint8

B, T, V, E, H = 4096, 32, 10000, 300, 300
NCORES = 8
BC = B // NCORES          # 512 batch rows per core
M = BC // 128             # 4 m-tiles
GCOLS = 1200              # gates: [i f o | g], sigmoid on 0:900, tanh on 900:1200
CHUNKS = [(0, 512), (512, 1024), (1024, 1200)]
EH = 640                  # [e(300) | h(300) | 1 | pad] -> 5 k-splits of 128
KS = [(0, 128), (128, 256), (256, 384), (384, 512), (512, 601)]
HL = 384                  # h_last padded: [h(300) | 1 | 0-pad]

Sig = mybir.ActivationFunctionType.Sigmoid
Tanh = mybir.ActivationFunctionType.Tanh
Mult = mybir.AluOpType.mult
Add = mybir.AluOpType.add

STEP_TIMES = []  # (step, engine, sim_time) records when KERNEL_PROBE=1


def build_bass(amt, cap_tiles):
    """amt[t] = number of active m-tiles at step t (non-increasing).
    cap_tiles[t] = tuple of m-tiles that may capture h_last at step t."""
    probe = bool(int(os.environ.get("KERNEL_PROBE", "0")))
    if probe:
        from concourse.bass_interp import add_callback2
    nc = bacc.Bacc("TRN2", target_bir_lowering=False, debug=False,
                   num_devices=NCORES)

    embw_d = nc.dram_tensor("embw", [V, E], bf16, kind="ExternalInput")
    ident_d = nc.dram_tensor("ident", [128, 128], bf16, kind="ExternalInput")
    wcomb_d = nc.dram_tensor("wcomb", [601, GCOLS], bf16, kind="ExternalInput")
    wc_d = nc.dram_tensor("wc", [HL, 2], bf16, kind="ExternalInput")
    idx_d = nc.dram_tensor("idx", [128, T, M], i32, kind="ExternalInput")
    mask_d = nc.dram_tensor("mask", [128, T, M], u8, kind="ExternalInput")
    maskf_d = nc.dram_tensor("maskf", [128, T, M], bf16, kind="ExternalInput")
    out_d = nc.dram_tensor("out", [BC, 2], f32, kind="ExternalOutput")

    with tile.TileContext(nc, num_cores=NCORES) as tc:
        with (
            tc.tile_pool(name="const", bufs=1) as cpool,
            tc.tile_pool(name="state", bufs=1) as spool,
            tc.tile_pool(name="eht", bufs=3) as tpool,
            tc.tile_pool(name="act", bufs=3) as apool,
            tc.tile_pool(name="psumA", bufs=2, space="PSUM") as ppoolA,
            tc.tile_pool(name="psumT", bufs=2, space="PSUM") as ppoolT,
        ):
            # ---------- constants ----------
            # idx first: the prologue gathers (and everything downstream)
            # wait on it. The first two steps' indices load as a tiny DMA so
            # gathers start ~1us earlier; loads alternate SP/ACT HWDGE queues.
            idx_sb = cpool.tile([128, T, M], i32, tag="idx")
            nc.sync.dma_start(out=idx_sb[:, 0:2, :], in_=idx_d[:, 0:2, :])
            nc.sync.dma_start(out=idx_sb[:, 2:T, :], in_=idx_d[:, 2:T, :])
            ident = cpool.tile([128, 128], bf16, tag="ident")
            nc.scalar.dma_start(out=ident[:, :], in_=ident_d[:, :])
            wcomb_sb = []
            for k, (d0, d1) in enumerate(KS):
                t_ = cpool.tile([d1 - d0, GCOLS], bf16, tag=f"wcomb{k}")
                eng = nc.sync if k % 2 == 0 else nc.scalar
                eng.dma_start(out=t_[:, :], in_=wcomb_d[d0:d1, :])
                wcomb_sb.append(t_)
            wc_sb = []
            for k in range(3):
                t_ = cpool.tile([128, 2], bf16, tag=f"wc{k}")
                eng = nc.sync if k % 2 == 0 else nc.scalar
                eng.dma_start(out=t_[:, :], in_=wc_d[k * 128:(k + 1) * 128, :])
                wc_sb.append(t_)
            mask_sb = cpool.tile([128, T, M], u8, tag="mask")
            nc.scalar.dma_start(out=mask_sb[:, :, :], in_=mask_d[:, :, :])
            maskf_sb = cpool.tile([128, T, M], bf16, tag="maskf")
            nc.scalar.dma_start(out=maskf_sb[:, :, :], in_=maskf_d[:, :, :])

            # ---------- state ----------
            # eh[p, m, buf, :] = [e_t(300) | h_{t-1}(300) | 1 | 0-pad(39)]
            eh = spool.tile([128, M, 2, EH], bf16, tag="eh")
            c_sb = spool.tile([128, M, H], bf16, tag="c")
            h_last = spool.tile([128, M, HL], bf16, tag="hlast")

            # ---------- prologue: gathers first (Pool is in-order), then
            # memsets for the h/ones/pad regions on DVE, off the gather path
            for t in range(2):
                if t < T and amt[t] > 0:
                    for m in range(amt[t]):
                        nc.gpsimd.indirect_dma_start(
                            out=eh[:, m, t % 2, 0:E],
                            out_offset=None,
                            in_=embw_d[:, :],
                            in_offset=bass.IndirectOffsetOnAxis(
                                ap=idx_sb[:, t, m:m + 1], axis=0),
                        )
            nc.vector.memset(eh[:, :, :, 300:600], 0.0)
            nc.vector.memset(eh[:, :, :, 600:601], 1.0)
            nc.vector.memset(eh[:, :, :, 601:640], 0.0)
            nc.vector.memset(c_sb[:, :, :], 0.0)
            nc.vector.memset(h_last[:, :, :], 0.0)
            nc.vector.memset(h_last[:, :, 300:301], 1.0)

            # ---------- recurrence ----------
            # Early/late split: transposes j0/j1 and matmul k-passes 0/1 touch
            # only the gathered embedding columns (dims 0..255), so they run a
            # step ahead, off the h -> gates critical chain. j2..j4 / k2..k4
            # need h_{t-1} (dims 256..601 overlap h at 300..600).
            ehTD, trpD, gpsD = {}, {}, {}

            # logits for a tile can run as soon as its last h_last capture
            # has been flushed (tiles die at different steps); only the
            # longest-lived tile's logits remain after the loop.
            lsb = spool.tile([128, M, 2], f32, tag="lsb")
            t_done = {}
            for tt in range(T):
                for mm in cap_tiles[tt]:
                    t_done[mm] = tt + 1
            logits_done = set()

            def emit_logits(m):
                logits_done.add(m)
                hlT = tpool.tile([128, 3, 128], bf16, tag="hlT")
                trp3 = ppoolT.tile([128, 5 * 128], bf16, tag="trp")
                for j in range(3):
                    nc.tensor.transpose(
                        out=trp3[:, j * 128:(j + 1) * 128],
                        in_=h_last[:, m, j * 128:(j + 1) * 128],
                        identity=ident[:, :])
                for j in range(3):
                    nc.vector.tensor_copy(hlT[:, j, :],
                                          trp3[:, j * 128:(j + 1) * 128])
                lp = ppoolA.tile([128, GCOLS], f32, tag="gates")
                for k in range(3):
                    nc.tensor.matmul(
                        lp[:, 0:2],
                        lhsT=hlT[:, k, :],
                        rhs=wc_sb[k][:, :],
                        start=(k == 0),
                        stop=(k == 2),
                    )
                nc.vector.tensor_copy(lsb[:, m, :], lp[:, 0:2])
                nc.sync.dma_start(out=out_d[m * 128:(m + 1) * 128, :],
                                  in_=lsb[:, m, :])

            def early(t, m):
                """Allocate next-step tiles for (t, m); emit e-only transposes,
                drains, and k0/k1 gate passes. Reads eh[:, m, t%2, 0:256]."""
                b = t % 2
                trp = ppoolT.tile([128, 5 * 128], bf16, tag="trp")
                ehT = tpool.tile([128, 5, 128], bf16, tag=f"ehT{m}")
                ehTD[m], trpD[m] = ehT, trp
                for j in (0, 1):
                    nc.tensor.transpose(
                        out=trp[:, j * 128:(j + 1) * 128],
                        in_=eh[:, m, b, j * 128:(j + 1) * 128],
                        identity=ident[:, :])
                nc.vector.tensor_copy(ehT[:, 0, :], trp[:, 0:128])
                nc.vector.tensor_copy(ehT[:, 1, :], trp[:, 128:256])
                gps = ppoolA.tile([128, GCOLS], f32, tag="gates")
                gpsD[m] = gps
                for k in (0, 1):
                    for (c0, c1) in CHUNKS:
                        nc.tensor.matmul(
                            gps[:, c0:c1], lhsT=ehT[:, k, :],
                            rhs=wcomb_sb[k][:, c0:c1],
                            start=(k == 0), stop=False)

            for m in range(amt[0]):
                early(0, m)

            pending_cap = []
            for t in range(T):
                a = amt[t]
                if a == 0:
                    break
                buf, nbuf = t % 2, (t + 1) % 2

                def finish(m, t=t, buf=buf, nbuf=nbuf):
                    """Deferred tail of tile m's step: tanh(c), h write.
                    Emitted one m-slot late so ACT's gate activations (which
                    free the gates PSUM for PE) are never queued behind it."""
                    tch = apool.tile([128, H], bf16, tag=f"tch{m % 2}")
                    nc.scalar.activation(tch[:, :], c_sb[:, m, :], Tanh)
                    ifo = ifo_tiles[m]
                    nc.vector.tensor_tensor(
                        out=eh[:, m, nbuf, 300:600], in0=ifo[:, 600:900],
                        in1=tch[:, :], op=Mult)
                    if m in cap_tiles[t]:
                        pending_cap.append((t, m, nbuf))

                def flush_caps_prev(a=a):
                    # h_t stays valid in its parity buffer for two steps, so
                    # the capture can run a step late, off the DVE chain.
                    # In the latency-bound tail it runs as arithmetic on the
                    # otherwise-idle Pool engine instead of DVE.
                    while pending_cap:
                        ct, cm, cb = pending_cap.pop(0)
                        if a <= 2:
                            capd = apool.tile([128, H], bf16, tag="capd")
                            nc.gpsimd.tensor_tensor(
                                out=capd[:, :], in0=eh[:, cm, cb, 300:600],
                                in1=h_last[:, cm, 0:300],
                                op=mybir.AluOpType.subtract)
                            nc.gpsimd.tensor_tensor(
                                out=capd[:, :], in0=capd[:, :],
                                in1=maskf_sb[:, ct, cm:cm + 1].to_broadcast(
                                    [128, H]),
                                op=Mult)
                            nc.gpsimd.tensor_tensor(
                                out=h_last[:, cm, 0:300],
                                in0=h_last[:, cm, 0:300], in1=capd[:, :],
                                op=Add)
                        else:
                            nc.vector.copy_predicated(
                                out=h_last[:, cm, 0:300],
                                mask=mask_sb[:, ct, cm:cm + 1].to_broadcast(
                                    [128, H]),
                                data=eh[:, cm, cb, 300:600],
                            )

                ifo_tiles = {}
                for m in range(a):
                    ehT, trp, gps = ehTD[m], trpD[m], gpsD[m]
                    if m == a - 1:
                        flush_caps_prev()
                        for dm in range(M):
                            if t_done.get(dm) == t and dm not in logits_done:
                                emit_logits(dm)
                    # late transposes (need h_{t-1}) + drains + k2..k4
                    for j in (2, 3, 4):
                        nc.tensor.transpose(
                            out=trp[:, j * 128:(j + 1) * 128],
                            in_=eh[:, m, buf, j * 128:(j + 1) * 128],
                            identity=ident[:, :])
                    nc.vector.tensor_copy(ehT[:, 2, :], trp[:, 256:384])
                    nc.vector.tensor_copy(ehT[:, 3, :], trp[:, 384:512])
                    nc.vector.tensor_copy(ehT[:, 4, :], trp[:, 512:640])
                    for k in (2, 3, 4):
                        d0, d1 = KS[k]
                        kr = d1 - d0
                        for (c0, c1) in CHUNKS:
                            nc.tensor.matmul(
                                gps[:, c0:c1], lhsT=ehT[0:kr, k, :],
                                rhs=wcomb_sb[k][:, c0:c1],
                                start=False, stop=(k == 4))
                    if probe:
                        add_callback2(
                            nc.tensor,
                            (lambda s, i, t=t, m=m: STEP_TIMES.append(
                                (t, f"PE{m}", s.time, s.pe_busy_start))),
                            ins=[gps[:, 0:1]])
                    # prefetch the gather for step t+2 into this buffer
                    if t + 2 < T and m < amt[t + 2]:
                        nc.gpsimd.indirect_dma_start(
                            out=eh[:, m, buf, 0:E],
                            out_offset=None,
                            in_=embw_d[:, :],
                            in_offset=bass.IndirectOffsetOnAxis(
                                ap=idx_sb[:, t + 2, m:m + 1], axis=0),
                        )
                    # next step's early work (e-part is already gathered)
                    if t + 1 < T and m < amt[t + 1]:
                        early(t + 1, m)
                    ifo = apool.tile([128, 900], bf16, tag=f"ifo{m % 3}")
                    ifo_tiles[m] = ifo
                    gt = apool.tile([128, H], bf16, tag=f"gt{m % 2}")
                    if a <= 2:
                        # i,f first (one call), then g: unblocks the DVE c
                        # chain earliest; o (only needed for h) goes last
                        nc.scalar.activation(ifo[:, 0:600], gps[:, 0:600], Sig)
                        nc.scalar.activation(gt[:, :], gps[:, 900:1200], Tanh)
                        nc.scalar.activation(ifo[:, 600:900], gps[:, 600:900], Sig)
                    else:
                        nc.scalar.activation(ifo[:, :], gps[:, 0:900], Sig)
                        nc.scalar.activation(gt[:, :], gps[:, 900:1200], Tanh)
                    t1 = apool.tile([128, H], bf16, tag=f"t1{m % 2}")
                    nc.vector.tensor_tensor(
                        out=t1[:, :], in0=ifo[:, 300:600], in1=c_sb[:, m, :],
                        op=Mult)
                    nc.vector.tensor_tensor(
                        out=c_sb[:, m, :], in0=ifo[:, 0:300], in1=gt[:, :],
                        op=Mult)
                    nc.vector.tensor_tensor(
                        out=c_sb[:, m, :], in0=c_sb[:, m, :], in1=t1[:, :],
                        op=Add)
                    if m >= 1:
                        finish(m - 1)
                finish(a - 1)
                if probe:
                    add_callback2(
                        nc.vector,
                        (lambda s, i, t=t: STEP_TIMES.append(
                            (t, "h0", s.time, s.pe_busy_start))),
                        ins=[eh[:, 0, nbuf, 300:600]])

            while pending_cap:
                ct, cm, cb = pending_cap.pop(0)
                nc.vector.copy_predicated(
                    out=h_last[:, cm, 0:300],
                    mask=mask_sb[:, ct, cm:cm + 1].to_broadcast([128, H]),
                    data=eh[:, cm, cb, 300:600],
                )

            # ---------- logits for tiles still alive at the end ----------
            for m in range(M):
                if m not in logits_done:
                    emit_logits(m)

    nc.compile()
    return nc


_NC_CACHE = {}
LAST_RESULT = None
LAST_NC = None
LAST_IN_MAPS = None
LAST_ROWS = None


def _schedule(cap_len_by_core):
    """amt[t] (max over cores) and capture-tile sets (union over cores)."""
    amt = []
    cap_tiles = []
    for t in range(T):
        a = 0
        caps = set()
        for lens in cap_len_by_core:
            n = int((lens > t).sum())
            a = max(a, -(-n // 128))
            pos = np.nonzero(lens == t + 1)[0]
            if len(pos):
                caps.update(range(int(pos[0]) // 128, int(pos[-1]) // 128 + 1))
        amt.append(a)
        cap_tiles.append(tuple(sorted(caps)))
    return amt, cap_tiles


def _host_prep(inputs):
    import ml_dtypes
    bf = ml_dtypes.bfloat16

    cap = np.asarray(inputs["cap"]).astype(np.int32)
    cap_len = np.asarray(inputs["cap_len"]).astype(np.int64)
    embed_w = np.asarray(inputs["embed_w"], dtype=np.float32)
    W_ih = np.asarray(inputs["W_ih"], dtype=np.float32)
    W_hh = np.asarray(inputs["W_hh"], dtype=np.float32)
    b = (np.asarray(inputs["b_ih"], dtype=np.float32)
         + np.asarray(inputs["b_hh"], dtype=np.float32))
    cls_v = np.asarray(inputs["cls_v"], dtype=np.float32)
    cls_g = np.asarray(inputs["cls_g"], dtype=np.float32)
    cls_b = np.asarray(inputs["cls_b"], dtype=np.float32)

    # gate order [i f o g]: sigmoid block 0:900, tanh block 900:1200
    perm = np.concatenate([np.arange(0, 300), np.arange(300, 600),
                           np.arange(900, 1200), np.arange(600, 900)])
    wcomb = np.zeros((601, GCOLS), np.float32)
    wcomb[0:300] = W_ih[perm].T
    wcomb[300:600] = W_hh[perm].T
    wcomb[600] = b[perm]
    Wc = cls_g * cls_v / np.linalg.norm(cls_v, axis=1, keepdims=True)  # [2,300]
    wc = np.zeros((HL, 2), np.float32)
    wc[0:300] = Wc.T
    wc[300] = cls_b

    # length-descending sort, dealt round-robin across cores
    order = np.argsort(-cap_len, kind="stable")
    rows = [order[c::NCORES] for c in range(NCORES)]   # each [512], desc lens
    lens_by_core = [cap_len[r] for r in rows]
    amt, cap_tiles = _schedule(lens_by_core)

    shared = {
        "embw": embed_w.astype(bf),
        "wcomb": wcomb.astype(bf),
        "wc": wc.astype(bf),
        "ident": np.eye(128, dtype=np.float32).astype(bf),
    }
    in_maps = []
    for c in range(NCORES):
        capc = cap[rows[c]]                            # [512, 32]
        lenc = cap_len[rows[c]]                        # [512]
        idx = np.ascontiguousarray(
            capc.reshape(M, 128, T).transpose(1, 2, 0)).astype(np.int32)
        lm = lenc.reshape(M, 128).T                    # [128, M]
        mask = (lm[:, None, :] - 1 == np.arange(T)[None, :, None]).astype(np.uint8)
        in_maps.append(dict(shared, idx=idx, mask=np.ascontiguousarray(mask),
                            maskf=np.ascontiguousarray(mask).astype(bf)))
    return amt, cap_tiles, in_maps, rows


def kernel(**inputs) -> np.ndarray:
    global LAST_RESULT, LAST_NC, LAST_IN_MAPS, LAST_ROWS
    amt, cap_tiles, in_maps, rows = _host_prep(inputs)
    key = (tuple(amt), tuple(cap_tiles))
    if key not in _NC_CACHE:
        _NC_CACHE[key] = build_bass(amt, cap_tiles)
    nc = _NC_CACHE[key]
    LAST_NC, LAST_IN_MAPS, LAST_ROWS = nc, in_maps, rows
    trace = bool(int(os.environ.get("KERNEL_TRACE", "0")))
    res = run_bass_kernel_spmd(nc, in_maps, core_ids=list(range(NCORES)),
                               trace=trace)
    LAST_RESULT = res
    out = np.empty((B, 2), np.float32)
    for c in range(NCORES):
        out[rows[c]] = res.results[c]["out"].astype(np.float32)
    return out
